# revision 18
# baseline (speedup 1.0000x reference)
"""Trainium2 Bass kernel for nn_BaseLUTLayer (soft-LUT layer).

Math: out[b,o] = sum_k lut[o,k] * prod_j (bit_j(k) ? x[b,m(o,j)] : 1-x[b,m(o,j)])

Strategy (sharded node x4, batch x2: each core owns 512 nodes x 512 batch):
  * odds transform: with w = 1-x, r = x/(1-x) = 1/w - 1:
        out[b,o] = (prod_j w_j) * H,   H = successive halving of lut with
        T_new[k'] = T_lo[k'] + r_j * T_hi[k']   (6 levels)
  * bf16 pipeline: DVE tensor_tensor bf16 runs 2x fp32; tensor_scalar 4x.
  * level 1 is 32 tensor_scalar slices (lut entries are per-partition
    scalars), split across DVE / ScalarE / GpSimd.
  * layout: nodes on SBUF partitions (4 chunks of 128), free dim = b=512.
    Host ships xT so the gather source table G[wire] = [w|r] needs no
    on-device transposes; G is bounced via DRAM and rows are fetched with
    dma_gather (2KB bf16 rows, slot-pair granularity, 2 SWDGE queues).
  * halving runs in place inside t1 (t_k = t1[:, 0:2^(5-k), :]) to fit SBUF.
"""

import numpy as np

import concourse.bass as bass
import concourse.mybir as mybir
from concourse import bacc
from concourse import tile
from concourse.bass_utils import run_bass_kernel_spmd

P = 128
IN = 1024
OUT = 2048
NB = 6
B_FULL = 1024
N_CORES = 8
NODE_SHARDS = 4
BATCH_SHARDS = 2
NODES_PER_CORE = OUT // NODE_SHARDS  # 512
NCHUNK = NODES_PER_CORE // P  # 4
B = B_FULL // BATCH_SHARDS  # 512
F32 = mybir.dt.float32
BF16 = mybir.dt.bfloat16
I16 = mybir.dt.int16
MULT = mybir.AluOpType.mult
ADD = mybir.AluOpType.add
MAX = mybir.AluOpType.max
# clamp x <= 1 - 2^-18 so r = x/(1-x) <= 2^18; bf16 range is fp32-like
WMIN = float(2.0**-18)


def build_program():
    nc = bacc.Bacc(
        "TRN2", target_bir_lowering=False, debug=False, num_swdge_queues=2
    )

    IH = IN // P  # 8
    xt = nc.dram_tensor("xt", [P, IH, B], F32, kind="ExternalInput").ap()
    gidx = nc.dram_tensor(
        "gidx", [P, NCHUNK * NB * P // 16], I16, kind="ExternalInput"
    ).ap()
    lutg = nc.dram_tensor("lutg", [P, NCHUNK, 64], F32, kind="ExternalInput").ap()
    outs = nc.dram_tensor("outs", [P, NCHUNK, B], F32, kind="ExternalOutput").ap()

    with tile.TileContext(nc) as tc:
        with (
            tc.tile_pool(name="consts", bufs=1) as consts,
            tc.tile_pool(name="dram", bufs=1, space="DRAM") as dpool,
        ):
            gidx_sb = consts.tile([P, NCHUNK * NB * P // 16], I16)
            nc.sync.dma_start(gidx_sb, gidx)
            lutg_sb = consts.tile([P, NCHUNK, 64], F32)
            nc.sync.dma_start(lutg_sb, lutg)
            neg_one = consts.tile([P, 1], F32)
            nc.vector.memset(neg_one, -1.0)

            # ---- prologue: build G[wire] = [w bf16 | r bf16] rows in DRAM,
            # pipelined in 4 double-ih slices under the xt load
            gd = dpool.tile([IN, 2 * B], BF16)
            gd_view = gd[:].rearrange("(p h) e -> p h e", h=IH)

            with tc.tile_pool(name="pro", bufs=1) as pro:
                xts = pro.tile([P, IH, B], F32)
                gsb = pro.tile([P, IH, 2 * B], BF16)
                wf = pro.tile([P, IH, B], F32)
                rf = pro.tile([P, IH, B], F32)
                HH = IH // 2
                h0 = slice(0, HH)
                h1 = slice(HH, IH)
                # split the load across both HWDGE engines (2 DMA queues)
                nc.sync.dma_start(xts[:, h0, :], xt[:, h0, :])
                nc.scalar.dma_start(xts[:, h1, :], xt[:, h1, :])
                # w = 1 - x, then clamp w >= WMIN. gpsimd helps only with the
                # (mult,add) form; its MAX ucode is pathologically slow.
                for eng, hs in ((nc.vector, h0), (nc.gpsimd, h1)):
                    eng.tensor_scalar(
                        out=wf[:, hs, :], in0=xts[:, hs, :],
                        scalar1=-1.0, scalar2=1.0, op0=MULT, op1=ADD,
                    )
                for hs in (h0, h1):
                    nc.vector.tensor_scalar(
                        out=wf[:, hs, :], in0=wf[:, hs, :],
                        scalar1=WMIN, scalar2=1.0, op0=MAX, op1=MULT,
                    )
                # G w-half (bf16 cast)
                nc.scalar.copy(gsb[:, h0, 0:B], wf[:, h0, :])
                nc.vector.reciprocal_approx_fast(rf[:, h0, :], wf[:, h0, :])
                nc.vector.tensor_copy(gsb[:, h1, 0:B], wf[:, h1, :])
                nc.vector.reciprocal_approx_fast(rf[:, h1, :], wf[:, h1, :])
                # r = 1/w - 1  (bf16 out)
                nc.scalar.activation(
                    gsb[:, h0, B : 2 * B], rf[:, h0, :],
                    mybir.ActivationFunctionType.Identity,
                    bias=neg_one, scale=1.0,
                )
                nc.vector.tensor_scalar(
                    out=gsb[:, h1, B : 2 * B], in0=rf[:, h1, :],
                    scalar1=1.0, scalar2=-1.0, op0=MULT, op1=ADD,
                )
                # G store split across both HWDGE engines too
                nc.scalar.dma_start(gd_view[:, h0, :], gsb[:, h0, :])
                nc.sync.dma_start(gd_view[:, h1, :], gsb[:, h1, :])

            idx_cols = NB * P // 16  # 48 idx columns per chunk

            with (
                tc.tile_pool(name="zpool", bufs=3) as zpool,
                tc.tile_pool(name="t1pool", bufs=2) as t1pool,
                tc.tile_pool(name="prpool", bufs=2) as prpool,
                tc.tile_pool(name="spool", bufs=2) as spool,
            ):

                def gather(c):
                    # 3 slot-pair gathers so L1 (needs slot 0 only) starts
                    # before the whole chunk lands
                    z = zpool.tile([P, NB, 2 * B], BF16, tag="z")
                    c0 = c * idx_cols
                    pair_cols = idx_cols // 3  # 16 cols = 256 idxs
                    for pq in range(3):
                        nc.gpsimd.dma_gather(
                            out_ap=z[:, 2 * pq : 2 * pq + 2, :],
                            in_ap=gd[:],
                            idxs_ap=gidx_sb[
                                :, c0 + pq * pair_cols : c0 + (pq + 1) * pair_cols
                            ],
                            num_idxs=2 * P,
                            num_idxs_reg=2 * P,
                            elem_size=2 * B,
                            queue_num=pq % 2,
                        )
                    return z

                def bcast(r, n):
                    return r[:, None, :].broadcast_to([P, n, B])

                def compute(c, z):
                    r5 = z[:, 0, B : 2 * B]
                    r4 = z[:, 1, B : 2 * B]
                    r3 = z[:, 2, B : 2 * B]
                    r2 = z[:, 3, B : 2 * B]
                    r1 = z[:, 4, B : 2 * B]
                    r0 = z[:, 5, B : 2 * B]

                    # L1: t1[k'] = lut[k'] + r5 * lut[32+k']  (32 slices).
                    # DVE gets few slices (its op overhead is high and it owns
                    # the L2-L6 chain); Scalar packs slices back-to-back with
                    # no gaps; hi slices (16..31, feeding L2's mul) first.
                    t1 = t1pool.tile([P, 32, B], BF16, tag="t1")

                    def l1(eng_act, kp):
                        if eng_act == "sc":
                            nc.scalar.activation(
                                t1[:, kp, :], r5,
                                mybir.ActivationFunctionType.Identity,
                                bias=lutg_sb[:, c, kp : kp + 1],
                                scale=lutg_sb[:, c, 32 + kp : 33 + kp],
                            )
                        else:
                            eng = nc.vector if eng_act == "v" else nc.gpsimd
                            eng.tensor_scalar(
                                out=t1[:, kp, :], in0=r5,
                                scalar1=lutg_sb[:, c, 32 + kp : 33 + kp],
                                scalar2=lutg_sb[:, c, kp : kp + 1],
                                op0=MULT, op1=ADD,
                            )

                    for kp in range(26, 32):
                        l1("v", kp)
                    for kp in range(16, 26):
                        l1("sc", kp)
                    for kp in range(0, 8):
                        l1("sc", kp)
                    for kp in range(8, 16):
                        l1("g", kp)

                    # W = prod_j w_j (big pairwise mul on gpsimd; the wq
                    # muls are emitted late so Vector's in-order queue does
                    # not stall on the cross-engine wp dependency)
                    wp = spool.tile([P, 3, B], BF16, tag="wp")
                    nc.gpsimd.tensor_mul(wp, z[:, 0:5:2, 0:B], z[:, 1:6:2, 0:B])

                    pr = prpool.tile([P, 16, B], BF16, tag="pr")

                    # L2..L6: t_new = t_lo + r_j * t_hi, in place in t1.
                    # L2's mul in halves: DVE's own hi slices (26:32) first,
                    # Scalar's (16:26) land while the first half runs.
                    nc.vector.tensor_mul(
                        pr[:, 10:16, :], bcast(r4, 6), t1[:, 26:32, :]
                    )
                    nc.vector.tensor_mul(
                        pr[:, 0:10, :], bcast(r4, 10), t1[:, 16:26, :]
                    )
                    nc.vector.tensor_add(t1[:, 0:16, :], pr, t1[:, 0:16, :])

                    nc.vector.tensor_mul(pr[:, 0:8, :], bcast(r3, 8), t1[:, 8:16, :])
                    nc.vector.tensor_add(t1[:, 0:8, :], pr[:, 0:8, :], t1[:, 0:8, :])

                    nc.vector.tensor_mul(pr[:, 0:4, :], bcast(r2, 4), t1[:, 4:8, :])
                    nc.vector.tensor_add(t1[:, 0:4, :], pr[:, 0:4, :], t1[:, 0:4, :])

                    nc.vector.tensor_mul(pr[:, 0:2, :], bcast(r1, 2), t1[:, 2:4, :])
                    nc.vector.tensor_add(t1[:, 0:2, :], pr[:, 0:2, :], t1[:, 0:2, :])

                    nc.vector.tensor_mul(pr[:, 0:1, :], bcast(r0, 1), t1[:, 1:2, :])
                    nc.vector.tensor_add(t1[:, 0:1, :], pr[:, 0:1, :], t1[:, 0:1, :])

                    # final: out = W * t6 (fp32 out)
                    wq = spool.tile([P, B], BF16, tag="wq")
                    nc.vector.tensor_mul(wq, wp[:, 0, :], wp[:, 1, :])
                    nc.vector.tensor_mul(wq, wq, wp[:, 2, :])
                    ot = spool.tile([P, B], F32, tag="ot")
                    nc.vector.tensor_mul(ot, t1[:, 0, :], wq)
                    nc.sync.dma_start(outs[:, c, :], ot)

                # JIT gather order: gather completions share one DMA
                # semaphore, so compute(c) must precede gather(c+2) in issue
                # order or it waits on it.
                zs = {}
                zs[0] = gather(0)
                for c in range(NCHUNK):
                    compute(c, zs[c])
                    del zs[c]
                    if c + 1 < NCHUNK:
                        zs[c + 1] = gather(c + 1)

    nc.compile()
    return nc


_CACHE: dict = {}


def _program():
    if "nc" not in _CACHE:
        _CACHE["nc"] = build_program()
    return _CACHE["nc"]


def make_inputs(x, lut_table, mapping):
    """Host-side input prep: transpose x (layout only), per-core shards of
    batch, lut, and gather indices."""
    x = np.ascontiguousarray(x, dtype=np.float32)
    lut_table = np.ascontiguousarray(lut_table, dtype=np.float32)
    mapping = np.asarray(mapping)

    in_maps = []
    for core in range(N_CORES):
        nshard = core % NODE_SHARDS
        bshard = core // NODE_SHARDS
        # xt[p, h, b] = x[b0 + b, p*8+h]  (wire i = p*8+h -> G row i)
        xb = x[bshard * B : (bshard + 1) * B]  # [512, 1024]
        xt_arr = np.ascontiguousarray(xb.T.reshape(P, IN // P, B))

        o0 = nshard * NODES_PER_CORE
        m_core = mapping[o0 : o0 + NODES_PER_CORE]  # [512, 6]
        m3 = m_core.reshape(NCHUNK, P, NB)  # [cc, o_p, j]
        # slot s holds wire j = 5-s; row index in G = wire id directly
        tvals = np.transpose(m3[:, :, ::-1], (0, 2, 1)).reshape(-1)  # (cc, s, o_p)
        gidx16 = tvals.reshape(-1, 16).T.astype(np.int16)
        gidx_arr = np.ascontiguousarray(np.tile(gidx16, (P // 16, 1)))

        lut_core = lut_table[o0 : o0 + NODES_PER_CORE]
        lutg_arr = np.ascontiguousarray(
            lut_core.reshape(NCHUNK, P, 64).transpose(1, 0, 2)
        )
        in_maps.append({"xt": xt_arr, "gidx": gidx_arr, "lutg": lutg_arr})
    return in_maps


def assemble_output(results):
    """results: 8 dicts with 'outs' [128, 4, 512] -> full [1024, 2048]."""
    out = np.empty((B_FULL, OUT), dtype=np.float32)
    for core in range(N_CORES):
        nshard = core % NODE_SHARDS
        bshard = core // NODE_SHARDS
        arr = results[core]["outs"]  # [o_p, cc, b]
        b0 = bshard * B
        for cc in range(NCHUNK):
            o0 = nshard * NODES_PER_CORE + cc * P
            out[b0 : b0 + B, o0 : o0 + P] = arr[:, cc, :].T
    return out


def kernel_with_results(x, lut_table, mapping, **kwargs):
    nc = _program()
    in_maps = make_inputs(x, lut_table, mapping)
    res = run_bass_kernel_spmd(nc, in_maps, core_ids=list(range(N_CORES)), **kwargs)
    return assemble_output(res.results), res


def kernel(x, lut_table, mapping):
    out, _ = kernel_with_results(x, lut_table, mapping)
    return out


if __name__ == "__main__":
    rng = np.random.default_rng(0)
    x = rng.random((B_FULL, IN), dtype=np.float32)
    lut = rng.standard_normal((OUT, 64), dtype=np.float32)
    mp = rng.integers(0, IN, (OUT, NB), dtype=np.int32)
    out = kernel(x, lut, mp)
    print(out.shape, out.dtype)


# revision 19
# speedup vs baseline: 1.0358x; 1.0358x over previous
"""Trainium2 Bass kernel for nn_BaseLUTLayer (soft-LUT layer).

Math: out[b,o] = sum_k lut[o,k] * prod_j (bit_j(k) ? x[b,m(o,j)] : 1-x[b,m(o,j)])

Strategy (sharded node x4, batch x2: each core owns 512 nodes x 512 batch):
  * odds transform: with w = 1-x, r = x/(1-x) = 1/w - 1:
        out[b,o] = (prod_j w_j) * H,   H = successive halving of lut with
        T_new[k'] = T_lo[k'] + r_j * T_hi[k']   (6 levels)
  * bf16 pipeline: DVE tensor_tensor bf16 runs 2x fp32; tensor_scalar 4x.
  * level 1 is 32 tensor_scalar slices (lut entries are per-partition
    scalars), split across DVE / ScalarE / GpSimd.
  * layout: nodes on SBUF partitions (4 chunks of 128), free dim = b=512.
    Host ships xT so the gather source table G[wire] = [w|r] needs no
    on-device transposes; G is bounced via DRAM and rows are fetched with
    dma_gather (2KB bf16 rows, slot-pair granularity, 2 SWDGE queues).
  * halving runs in place inside t1 (t_k = t1[:, 0:2^(5-k), :]) to fit SBUF.
"""

import numpy as np

import concourse.bass as bass
import concourse.mybir as mybir
from concourse import bacc
from concourse import tile
from concourse.bass_utils import run_bass_kernel_spmd

P = 128
IN = 1024
OUT = 2048
NB = 6
B_FULL = 1024
N_CORES = 8
NODE_SHARDS = 4
BATCH_SHARDS = 2
NODES_PER_CORE = OUT // NODE_SHARDS  # 512
NCHUNK = NODES_PER_CORE // P  # 4
B = B_FULL // BATCH_SHARDS  # 512
F32 = mybir.dt.float32
BF16 = mybir.dt.bfloat16
I16 = mybir.dt.int16
MULT = mybir.AluOpType.mult
ADD = mybir.AluOpType.add
MAX = mybir.AluOpType.max
# clamp x <= 1 - 2^-18 so r = x/(1-x) <= 2^18; bf16 range is fp32-like
WMIN = float(2.0**-18)


def build_program():
    nc = bacc.Bacc(
        "TRN2", target_bir_lowering=False, debug=False, num_swdge_queues=2
    )

    IH = IN // P  # 8
    xt = nc.dram_tensor("xt", [P, IH, B], F32, kind="ExternalInput").ap()
    gidx = nc.dram_tensor(
        "gidx", [P, NCHUNK * NB * P // 16], I16, kind="ExternalInput"
    ).ap()
    lutg = nc.dram_tensor("lutg", [P, NCHUNK, 64], F32, kind="ExternalInput").ap()
    outs = nc.dram_tensor("outs", [P, NCHUNK, B], F32, kind="ExternalOutput").ap()

    with tile.TileContext(nc) as tc:
        with (
            tc.tile_pool(name="consts", bufs=1) as consts,
            tc.tile_pool(name="dram", bufs=1, space="DRAM") as dpool,
        ):
            gidx_sb = consts.tile([P, NCHUNK * NB * P // 16], I16)
            nc.sync.dma_start(gidx_sb, gidx)
            lutg_sb = consts.tile([P, NCHUNK, 64], F32)
            nc.sync.dma_start(lutg_sb, lutg)
            neg_one = consts.tile([P, 1], F32)
            nc.vector.memset(neg_one, -1.0)

            # ---- prologue: build G[wire] = [w bf16 | r bf16] rows in DRAM,
            # pipelined in 4 double-ih slices under the xt load
            gd = dpool.tile([IN, 2 * B], BF16)
            gd_view = gd[:].rearrange("(p h) e -> p h e", h=IH)

            with tc.tile_pool(name="pro", bufs=1) as pro:
                xts = pro.tile([P, IH, B], F32)
                gsb = pro.tile([P, IH, 2 * B], BF16)
                wf = pro.tile([P, IH, B], F32)
                rf = pro.tile([P, IH, B], F32)
                HH = IH // 2
                h0 = slice(0, HH)
                h1 = slice(HH, IH)
                # split the load across both HWDGE engines (2 DMA queues)
                nc.sync.dma_start(xts[:, h0, :], xt[:, h0, :])
                nc.scalar.dma_start(xts[:, h1, :], xt[:, h1, :])
                # w = 1 - x, then clamp w >= WMIN. gpsimd helps only with the
                # (mult,add) form; its MAX ucode is pathologically slow.
                for eng, hs in ((nc.vector, h0), (nc.gpsimd, h1)):
                    eng.tensor_scalar(
                        out=wf[:, hs, :], in0=xts[:, hs, :],
                        scalar1=-1.0, scalar2=1.0, op0=MULT, op1=ADD,
                    )
                for hs in (h0, h1):
                    nc.vector.tensor_scalar(
                        out=wf[:, hs, :], in0=wf[:, hs, :],
                        scalar1=WMIN, scalar2=1.0, op0=MAX, op1=MULT,
                    )
                # G w-half (bf16 cast)
                nc.scalar.copy(gsb[:, h0, 0:B], wf[:, h0, :])
                nc.vector.reciprocal_approx_fast(rf[:, h0, :], wf[:, h0, :])
                nc.vector.tensor_copy(gsb[:, h1, 0:B], wf[:, h1, :])
                nc.vector.reciprocal_approx_fast(rf[:, h1, :], wf[:, h1, :])
                # r = 1/w - 1  (bf16 out)
                nc.scalar.activation(
                    gsb[:, h0, B : 2 * B], rf[:, h0, :],
                    mybir.ActivationFunctionType.Identity,
                    bias=neg_one, scale=1.0,
                )
                nc.vector.tensor_scalar(
                    out=gsb[:, h1, B : 2 * B], in0=rf[:, h1, :],
                    scalar1=1.0, scalar2=-1.0, op0=MULT, op1=ADD,
                )
                # G store split across both HWDGE engines too
                nc.scalar.dma_start(gd_view[:, h0, :], gsb[:, h0, :])
                nc.sync.dma_start(gd_view[:, h1, :], gsb[:, h1, :])

            idx_cols = NB * P // 16  # 48 idx columns per chunk

            with (
                tc.tile_pool(name="zpool", bufs=3) as zpool,
                tc.tile_pool(name="t1pool", bufs=2) as t1pool,
                tc.tile_pool(name="prpool", bufs=2) as prpool,
                tc.tile_pool(name="spool", bufs=2) as spool,
            ):

                def gather(c):
                    # 2 half-gathers, one per SWDGE queue: consecutive
                    # gathers on the SAME queue serialize on DMA completion,
                    # so each queue is used once per chunk. The first half
                    # (slots 0-2) unlocks L1/L2 early.
                    z = zpool.tile([P, NB, 2 * B], BF16, tag="z")
                    c0 = c * idx_cols
                    half_cols = idx_cols // 2  # 24 cols = 384 idxs
                    for pq in range(2):
                        nc.gpsimd.dma_gather(
                            out_ap=z[:, 3 * pq : 3 * pq + 3, :],
                            in_ap=gd[:],
                            idxs_ap=gidx_sb[
                                :, c0 + pq * half_cols : c0 + (pq + 1) * half_cols
                            ],
                            num_idxs=3 * P,
                            num_idxs_reg=3 * P,
                            elem_size=2 * B,
                            queue_num=pq,
                        )
                    return z

                def bcast(r, n):
                    return r[:, None, :].broadcast_to([P, n, B])

                def compute(c, z):
                    r5 = z[:, 0, B : 2 * B]
                    r4 = z[:, 1, B : 2 * B]
                    r3 = z[:, 2, B : 2 * B]
                    r2 = z[:, 3, B : 2 * B]
                    r1 = z[:, 4, B : 2 * B]
                    r0 = z[:, 5, B : 2 * B]

                    # L1: t1[k'] = lut[k'] + r5 * lut[32+k']  (32 slices).
                    # DVE gets few slices (its op overhead is high and it owns
                    # the L2-L6 chain); Scalar packs slices back-to-back with
                    # no gaps; hi slices (16..31, feeding L2's mul) first.
                    t1 = t1pool.tile([P, 32, B], BF16, tag="t1")

                    def l1(eng_act, kp):
                        if eng_act == "sc":
                            nc.scalar.activation(
                                t1[:, kp, :], r5,
                                mybir.ActivationFunctionType.Identity,
                                bias=lutg_sb[:, c, kp : kp + 1],
                                scale=lutg_sb[:, c, 32 + kp : 33 + kp],
                            )
                        else:
                            eng = nc.vector if eng_act == "v" else nc.gpsimd
                            eng.tensor_scalar(
                                out=t1[:, kp, :], in0=r5,
                                scalar1=lutg_sb[:, c, 32 + kp : 33 + kp],
                                scalar2=lutg_sb[:, c, kp : kp + 1],
                                op0=MULT, op1=ADD,
                            )

                    for kp in range(26, 32):
                        l1("v", kp)
                    for kp in range(16, 26):
                        l1("sc", kp)
                    for kp in range(0, 8):
                        l1("sc", kp)
                    for kp in range(8, 16):
                        l1("g", kp)

                    # W = prod_j w_j (big pairwise mul on gpsimd; the wq
                    # muls are emitted late so Vector's in-order queue does
                    # not stall on the cross-engine wp dependency)
                    wp = spool.tile([P, 3, B], BF16, tag="wp")
                    nc.gpsimd.tensor_mul(wp, z[:, 0:5:2, 0:B], z[:, 1:6:2, 0:B])

                    pr = prpool.tile([P, 16, B], BF16, tag="pr")

                    # L2..L6: t_new = t_lo + r_j * t_hi, in place in t1.
                    # L2's mul in halves: DVE's own hi slices (26:32) first,
                    # Scalar's (16:26) land while the first half runs.
                    nc.vector.tensor_mul(
                        pr[:, 10:16, :], bcast(r4, 6), t1[:, 26:32, :]
                    )
                    nc.vector.tensor_mul(
                        pr[:, 0:10, :], bcast(r4, 10), t1[:, 16:26, :]
                    )
                    nc.vector.tensor_add(t1[:, 0:16, :], pr, t1[:, 0:16, :])

                    nc.vector.tensor_mul(pr[:, 0:8, :], bcast(r3, 8), t1[:, 8:16, :])
                    nc.vector.tensor_add(t1[:, 0:8, :], pr[:, 0:8, :], t1[:, 0:8, :])

                    nc.vector.tensor_mul(pr[:, 0:4, :], bcast(r2, 4), t1[:, 4:8, :])
                    nc.vector.tensor_add(t1[:, 0:4, :], pr[:, 0:4, :], t1[:, 0:4, :])

                    nc.vector.tensor_mul(pr[:, 0:2, :], bcast(r1, 2), t1[:, 2:4, :])
                    nc.vector.tensor_add(t1[:, 0:2, :], pr[:, 0:2, :], t1[:, 0:2, :])

                    nc.vector.tensor_mul(pr[:, 0:1, :], bcast(r0, 1), t1[:, 1:2, :])
                    nc.vector.tensor_add(t1[:, 0:1, :], pr[:, 0:1, :], t1[:, 0:1, :])

                    # final: out = W * t6 (fp32 out)
                    wq = spool.tile([P, B], BF16, tag="wq")
                    nc.vector.tensor_mul(wq, wp[:, 0, :], wp[:, 1, :])
                    nc.vector.tensor_mul(wq, wq, wp[:, 2, :])
                    ot = spool.tile([P, B], F32, tag="ot")
                    nc.vector.tensor_mul(ot, t1[:, 0, :], wq)
                    nc.sync.dma_start(outs[:, c, :], ot)

                # JIT gather order: gather completions share one DMA
                # semaphore, so compute(c) must precede gather(c+2) in issue
                # order or it waits on it.
                zs = {}
                zs[0] = gather(0)
                for c in range(NCHUNK):
                    compute(c, zs[c])
                    del zs[c]
                    if c + 1 < NCHUNK:
                        zs[c + 1] = gather(c + 1)

    nc.compile()
    return nc


_CACHE: dict = {}


def _program():
    if "nc" not in _CACHE:
        _CACHE["nc"] = build_program()
    return _CACHE["nc"]


def make_inputs(x, lut_table, mapping):
    """Host-side input prep: transpose x (layout only), per-core shards of
    batch, lut, and gather indices."""
    x = np.ascontiguousarray(x, dtype=np.float32)
    lut_table = np.ascontiguousarray(lut_table, dtype=np.float32)
    mapping = np.asarray(mapping)

    in_maps = []
    for core in range(N_CORES):
        nshard = core % NODE_SHARDS
        bshard = core // NODE_SHARDS
        # xt[p, h, b] = x[b0 + b, p*8+h]  (wire i = p*8+h -> G row i)
        xb = x[bshard * B : (bshard + 1) * B]  # [512, 1024]
        xt_arr = np.ascontiguousarray(xb.T.reshape(P, IN // P, B))

        o0 = nshard * NODES_PER_CORE
        m_core = mapping[o0 : o0 + NODES_PER_CORE]  # [512, 6]
        m3 = m_core.reshape(NCHUNK, P, NB)  # [cc, o_p, j]
        # slot s holds wire j = 5-s; row index in G = wire id directly
        tvals = np.transpose(m3[:, :, ::-1], (0, 2, 1)).reshape(-1)  # (cc, s, o_p)
        gidx16 = tvals.reshape(-1, 16).T.astype(np.int16)
        gidx_arr = np.ascontiguousarray(np.tile(gidx16, (P // 16, 1)))

        lut_core = lut_table[o0 : o0 + NODES_PER_CORE]
        lutg_arr = np.ascontiguousarray(
            lut_core.reshape(NCHUNK, P, 64).transpose(1, 0, 2)
        )
        in_maps.append({"xt": xt_arr, "gidx": gidx_arr, "lutg": lutg_arr})
    return in_maps


def assemble_output(results):
    """results: 8 dicts with 'outs' [128, 4, 512] -> full [1024, 2048]."""
    out = np.empty((B_FULL, OUT), dtype=np.float32)
    for core in range(N_CORES):
        nshard = core % NODE_SHARDS
        bshard = core // NODE_SHARDS
        arr = results[core]["outs"]  # [o_p, cc, b]
        b0 = bshard * B
        for cc in range(NCHUNK):
            o0 = nshard * NODES_PER_CORE + cc * P
            out[b0 : b0 + B, o0 : o0 + P] = arr[:, cc, :].T
    return out


def kernel_with_results(x, lut_table, mapping, **kwargs):
    nc = _program()
    in_maps = make_inputs(x, lut_table, mapping)
    res = run_bass_kernel_spmd(nc, in_maps, core_ids=list(range(N_CORES)), **kwargs)
    return assemble_output(res.results), res


def kernel(x, lut_table, mapping):
    out, _ = kernel_with_results(x, lut_table, mapping)
    return out


if __name__ == "__main__":
    rng = np.random.default_rng(0)
    x = rng.random((B_FULL, IN), dtype=np.float32)
    lut = rng.standard_normal((OUT, 64), dtype=np.float32)
    mp = rng.integers(0, IN, (OUT, NB), dtype=np.int32)
    out = kernel(x, lut, mp)
    print(out.shape, out.dtype)


# revision 20
# speedup vs baseline: 1.5503x; 1.4968x over previous
"""Trainium2 Bass kernel for nn_BaseLUTLayer (soft-LUT layer).

Math: out[b,o] = sum_k lut[o,k] * prod_j (bit_j(k) ? x[b,m(o,j)] : 1-x[b,m(o,j)])

Strategy (sharded node x4, batch x2: each core owns 512 nodes x 512 batch):
  * odds transform: with w = 1-x, r = x/(1-x) = 1/w - 1:
        out[b,o] = (prod_j w_j) * H,   H = successive halving of lut with
        T_new[k'] = T_lo[k'] + r_j * T_hi[k']   (6 levels)
  * bf16 pipeline: DVE tensor_tensor bf16 runs 2x fp32; tensor_scalar 4x.
  * level 1 is 32 tensor_scalar slices (lut entries are per-partition
    scalars), split across DVE / ScalarE / GpSimd.
  * layout: nodes on SBUF partitions (4 chunks of 128), free dim = b=512.
    Host ships xT so the gather source table G[wire] = [w|r] needs no
    on-device transposes; G is bounced via DRAM and rows are fetched with
    dma_gather (2KB bf16 rows, slot-pair granularity, 2 SWDGE queues).
  * halving runs in place inside t1 (t_k = t1[:, 0:2^(5-k), :]) to fit SBUF.
"""

import numpy as np

import concourse.bass as bass
import concourse.mybir as mybir
from concourse import bacc
from concourse import tile
from concourse.bass_utils import run_bass_kernel_spmd

P = 128
IN = 1024
OUT = 2048
NB = 6
B_FULL = 1024
N_CORES = 8
NODE_SHARDS = 4
BATCH_SHARDS = 2
NODES_PER_CORE = OUT // NODE_SHARDS  # 512
NCHUNK = NODES_PER_CORE // P  # 4
B = B_FULL // BATCH_SHARDS  # 512
F32 = mybir.dt.float32
BF16 = mybir.dt.bfloat16
I16 = mybir.dt.int16
MULT = mybir.AluOpType.mult
ADD = mybir.AluOpType.add
MAX = mybir.AluOpType.max
# clamp x <= 1 - 2^-18 so r = x/(1-x) <= 2^18; bf16 range is fp32-like
WMIN = float(2.0**-18)


def build_program():
    nc = bacc.Bacc(
        "TRN2", target_bir_lowering=False, debug=False, num_swdge_queues=2
    )

    IH = IN // P  # 8
    xt = nc.dram_tensor("xt", [P, IH, B], F32, kind="ExternalInput").ap()
    gidx = nc.dram_tensor(
        "gidx", [P, NCHUNK * NB * P // 16], I16, kind="ExternalInput"
    ).ap()
    lutg = nc.dram_tensor("lutg", [P, NCHUNK, 64], F32, kind="ExternalInput").ap()
    outs = nc.dram_tensor("outs", [P, NCHUNK, B], F32, kind="ExternalOutput").ap()

    with tile.TileContext(nc) as tc:
        with (
            tc.tile_pool(name="consts", bufs=1) as consts,
            tc.tile_pool(name="dram", bufs=1, space="DRAM") as dpool,
        ):
            gidx_sb = consts.tile([P, NCHUNK * NB * P // 16], I16)
            nc.sync.dma_start(gidx_sb, gidx)
            lutg_sb = consts.tile([P, NCHUNK, 64], F32)
            nc.sync.dma_start(lutg_sb, lutg)
            neg_one = consts.tile([P, 1], F32)
            nc.vector.memset(neg_one, -1.0)

            # ---- prologue: build G[wire] = [w bf16 | r bf16] rows in DRAM,
            # pipelined in 4 double-ih slices under the xt load
            gd = dpool.tile([IN, 2 * B], BF16)
            gd_view = gd[:].rearrange("(p h) e -> p h e", h=IH)

            with tc.tile_pool(name="pro", bufs=1) as pro:
                xts = pro.tile([P, IH, B], F32)
                gsb = pro.tile([P, IH, 2 * B], BF16)
                wf = pro.tile([P, IH, B], F32)
                rf = pro.tile([P, IH, B], F32)
                HH = IH // 2
                h0 = slice(0, HH)
                h1 = slice(HH, IH)
                nc.sync.dma_start(xts, xt)
                # w = 1 - x, then clamp w >= WMIN. gpsimd helps only with the
                # (mult,add) form; its MAX ucode is pathologically slow.
                for eng, hs in ((nc.vector, h0), (nc.gpsimd, h1)):
                    eng.tensor_scalar(
                        out=wf[:, hs, :], in0=xts[:, hs, :],
                        scalar1=-1.0, scalar2=1.0, op0=MULT, op1=ADD,
                    )
                for hs in (h0, h1):
                    nc.vector.tensor_scalar(
                        out=wf[:, hs, :], in0=wf[:, hs, :],
                        scalar1=WMIN, scalar2=1.0, op0=MAX, op1=MULT,
                    )
                # G w-half (bf16 cast)
                nc.scalar.copy(gsb[:, h0, 0:B], wf[:, h0, :])
                nc.vector.reciprocal_approx_fast(rf[:, h0, :], wf[:, h0, :])
                nc.vector.tensor_copy(gsb[:, h1, 0:B], wf[:, h1, :])
                nc.vector.reciprocal_approx_fast(rf[:, h1, :], wf[:, h1, :])
                # r = 1/w - 1  (bf16 out)
                nc.scalar.activation(
                    gsb[:, h0, B : 2 * B], rf[:, h0, :],
                    mybir.ActivationFunctionType.Identity,
                    bias=neg_one, scale=1.0,
                )
                nc.vector.tensor_scalar(
                    out=gsb[:, h1, B : 2 * B], in0=rf[:, h1, :],
                    scalar1=1.0, scalar2=-1.0, op0=MULT, op1=ADD,
                )
                nc.sync.dma_start(gd_view[:, h0, :], gsb[:, h0, :])
                nc.sync.dma_start(gd_view[:, h1, :], gsb[:, h1, :])

            idx_cols = NB * P // 16  # 48 idx columns per chunk

            with (
                tc.tile_pool(name="zpool", bufs=3) as zpool,
                tc.tile_pool(name="t1pool", bufs=2) as t1pool,
                tc.tile_pool(name="prpool", bufs=2) as prpool,
                tc.tile_pool(name="spool", bufs=2) as spool,
            ):

                def gather(c):
                    # 3 slot-pair gathers so L1 (needs slot 0 only) starts
                    # before the whole chunk lands
                    z = zpool.tile([P, NB, 2 * B], BF16, tag="z")
                    c0 = c * idx_cols
                    pair_cols = idx_cols // 3  # 16 cols = 256 idxs
                    for pq in range(3):
                        nc.gpsimd.dma_gather(
                            out_ap=z[:, 2 * pq : 2 * pq + 2, :],
                            in_ap=gd[:],
                            idxs_ap=gidx_sb[
                                :, c0 + pq * pair_cols : c0 + (pq + 1) * pair_cols
                            ],
                            num_idxs=2 * P,
                            num_idxs_reg=2 * P,
                            elem_size=2 * B,
                            queue_num=pq % 2,
                        )
                    return z

                def bcast(r, n):
                    return r[:, None, :].broadcast_to([P, n, B])

                def compute(c, z):
                    r5 = z[:, 0, B : 2 * B]
                    r4 = z[:, 1, B : 2 * B]
                    r3 = z[:, 2, B : 2 * B]
                    r2 = z[:, 3, B : 2 * B]
                    r1 = z[:, 4, B : 2 * B]
                    r0 = z[:, 5, B : 2 * B]

                    # L1: t1[k'] = lut[k'] + r5 * lut[32+k']  (32 slices).
                    # DVE gets few slices (its op overhead is high and it owns
                    # the L2-L6 chain); Scalar packs slices back-to-back with
                    # no gaps; hi slices (16..31, feeding L2's mul) first.
                    t1 = t1pool.tile([P, 32, B], BF16, tag="t1")

                    def l1(eng_act, kp):
                        if eng_act == "sc":
                            nc.scalar.activation(
                                t1[:, kp, :], r5,
                                mybir.ActivationFunctionType.Identity,
                                bias=lutg_sb[:, c, kp : kp + 1],
                                scale=lutg_sb[:, c, 32 + kp : 33 + kp],
                            )
                        else:
                            eng = nc.vector if eng_act == "v" else nc.gpsimd
                            eng.tensor_scalar(
                                out=t1[:, kp, :], in0=r5,
                                scalar1=lutg_sb[:, c, 32 + kp : 33 + kp],
                                scalar2=lutg_sb[:, c, kp : kp + 1],
                                op0=MULT, op1=ADD,
                            )

                    for kp in range(24, 32):
                        l1("v", kp)
                    for kp in range(16, 24):
                        l1("sc", kp)
                    for kp in range(0, 8):
                        l1("sc", kp)
                    for kp in range(8, 16):
                        l1("g", kp)

                    # W = prod_j w_j
                    wp = spool.tile([P, 3, B], BF16, tag="wp")
                    nc.vector.tensor_mul(wp, z[:, 0:5:2, 0:B], z[:, 1:6:2, 0:B])
                    wq = spool.tile([P, B], BF16, tag="wq")
                    nc.vector.tensor_mul(wq, wp[:, 0, :], wp[:, 1, :])
                    nc.vector.tensor_mul(wq, wq, wp[:, 2, :])

                    pr = prpool.tile([P, 16, B], BF16, tag="pr")

                    # L2..L6: t_new = t_lo + r_j * t_hi, in place in t1.
                    # L2's mul in halves: DVE's own hi slices (26:32) first,
                    # Scalar's (16:26) land while the first half runs.
                    nc.vector.tensor_mul(
                        pr[:, 8:16, :], bcast(r4, 8), t1[:, 24:32, :]
                    )
                    nc.vector.tensor_mul(
                        pr[:, 0:8, :], bcast(r4, 8), t1[:, 16:24, :]
                    )
                    nc.vector.tensor_add(t1[:, 0:16, :], pr, t1[:, 0:16, :])

                    nc.vector.tensor_mul(pr[:, 0:8, :], bcast(r3, 8), t1[:, 8:16, :])
                    nc.vector.tensor_add(t1[:, 0:8, :], pr[:, 0:8, :], t1[:, 0:8, :])

                    nc.vector.tensor_mul(pr[:, 0:4, :], bcast(r2, 4), t1[:, 4:8, :])
                    nc.vector.tensor_add(t1[:, 0:4, :], pr[:, 0:4, :], t1[:, 0:4, :])

                    nc.vector.tensor_mul(pr[:, 0:2, :], bcast(r1, 2), t1[:, 2:4, :])
                    nc.vector.tensor_add(t1[:, 0:2, :], pr[:, 0:2, :], t1[:, 0:2, :])

                    nc.vector.tensor_mul(pr[:, 0:1, :], bcast(r0, 1), t1[:, 1:2, :])
                    nc.vector.tensor_add(t1[:, 0:1, :], pr[:, 0:1, :], t1[:, 0:1, :])

                    # final: out = W * t6 (fp32 out)
                    ot = spool.tile([P, B], F32, tag="ot")
                    nc.vector.tensor_mul(ot, t1[:, 0, :], wq)
                    nc.sync.dma_start(outs[:, c, :], ot)

                # JIT gather order: gather completions share one DMA
                # semaphore, so compute(c) must precede gather(c+2) in issue
                # order or it waits on it.
                zs = {}
                zs[0] = gather(0)
                for c in range(NCHUNK):
                    compute(c, zs[c])
                    del zs[c]
                    if c + 1 < NCHUNK:
                        zs[c + 1] = gather(c + 1)

    nc.compile()
    return nc


_CACHE: dict = {}


def _program():
    if "nc" not in _CACHE:
        _CACHE["nc"] = build_program()
    return _CACHE["nc"]


def make_inputs(x, lut_table, mapping):
    """Host-side input prep: transpose x (layout only), per-core shards of
    batch, lut, and gather indices."""
    x = np.ascontiguousarray(x, dtype=np.float32)
    lut_table = np.ascontiguousarray(lut_table, dtype=np.float32)
    mapping = np.asarray(mapping)

    in_maps = []
    for core in range(N_CORES):
        nshard = core % NODE_SHARDS
        bshard = core // NODE_SHARDS
        # xt[p, h, b] = x[b0 + b, p*8+h]  (wire i = p*8+h -> G row i)
        xb = x[bshard * B : (bshard + 1) * B]  # [512, 1024]
        xt_arr = np.ascontiguousarray(xb.T.reshape(P, IN // P, B))

        o0 = nshard * NODES_PER_CORE
        m_core = mapping[o0 : o0 + NODES_PER_CORE]  # [512, 6]
        m3 = m_core.reshape(NCHUNK, P, NB)  # [cc, o_p, j]
        # slot s holds wire j = 5-s; row index in G = wire id directly
        tvals = np.transpose(m3[:, :, ::-1], (0, 2, 1)).reshape(-1)  # (cc, s, o_p)
        gidx16 = tvals.reshape(-1, 16).T.astype(np.int16)
        gidx_arr = np.ascontiguousarray(np.tile(gidx16, (P // 16, 1)))

        lut_core = lut_table[o0 : o0 + NODES_PER_CORE]
        lutg_arr = np.ascontiguousarray(
            lut_core.reshape(NCHUNK, P, 64).transpose(1, 0, 2)
        )
        in_maps.append({"xt": xt_arr, "gidx": gidx_arr, "lutg": lutg_arr})
    return in_maps


def assemble_output(results):
    """results: 8 dicts with 'outs' [128, 4, 512] -> full [1024, 2048]."""
    out = np.empty((B_FULL, OUT), dtype=np.float32)
    for core in range(N_CORES):
        nshard = core % NODE_SHARDS
        bshard = core // NODE_SHARDS
        arr = results[core]["outs"]  # [o_p, cc, b]
        b0 = bshard * B
        for cc in range(NCHUNK):
            o0 = nshard * NODES_PER_CORE + cc * P
            out[b0 : b0 + B, o0 : o0 + P] = arr[:, cc, :].T
    return out


def kernel_with_results(x, lut_table, mapping, **kwargs):
    nc = _program()
    in_maps = make_inputs(x, lut_table, mapping)
    res = run_bass_kernel_spmd(nc, in_maps, core_ids=list(range(N_CORES)), **kwargs)
    return assemble_output(res.results), res


def kernel(x, lut_table, mapping):
    out, _ = kernel_with_results(x, lut_table, mapping)
    return out


if __name__ == "__main__":
    rng = np.random.default_rng(0)
    x = rng.random((B_FULL, IN), dtype=np.float32)
    lut = rng.standard_normal((OUT, 64), dtype=np.float32)
    mp = rng.integers(0, IN, (OUT, NB), dtype=np.int32)
    out = kernel(x, lut, mp)
    print(out.shape, out.dtype)
